# revision 1
# baseline (speedup 1.0000x reference)
"""Bonsai soft-decision-tree forward pass on 8 TRN2 NeuronCores.

Math (per batch row b):
    pp    = (x @ Z) / 64                      [B, 64]
    tb_j  = tanh(4 * pp @ T[j])               internal nodes j = 0..14
    I_0   = 1;  I_{2j+1} = I_j * 0.5*(1+tb_j);  I_{2j+2} = I_j * 0.5*(1-tb_j)
    score = sum_n I_n * (pp @ W[n]) * tanh(4 * pp @ V[n])   [B, 32]

Distribution: pure data parallel. The batch (100000 rows) is padded to
8 * 12544 and split across the 8 cores; Z/T/V/W are replicated.

Per-core layouts (prepared on host, all matmul contractions end up on
SBUF partitions):
    xT   [128, 4, Bc]  xT[p, c, b] = x[b, 128*c + p]
    Zs   [128, 4, 64]  Zs[p, c, d] = Z[128*c + p, d] / 64
    T4T  [64, 16]      (4*T).T zero-padded from 15 to 16 columns
    Wcm  [64, 1024]    W.transpose(1,2,0) (channel-major, node padded to 32)
    Vcm  [64, 1024]    (4*V).transpose(1,2,0), same packing
The kernel streams 128-row batch tiles: ppT = Zs.T @ xT chunks (PSUM
accum over 4 chunks), tree logits via one matmul, indicators via
sigmoid + a batched product recursion, wp/vp via two 512-wide matmuls
each, then tanh / multiplies / a segmented reduce over nodes.
"""

import numpy as np

# Problem constants (fixed by the reference).
INT_N = 15
TOT_N = 31
NF = 512
PD = 64
NCH = 32
BATCH = 100000
SIG = 4.0

N_CORES = 8
P = 128                       # partition count / batch tile rows
NODES = 32                    # node dim padded (node 31 is zero weights)
FREE = NODES * NCH            # 1024: elementwise width per batch row
B_RAW = BATCH // N_CORES      # 12500
TILES_FULL = (B_RAW + P - 1) // P   # 98
B_CORE = TILES_FULL * P       # 12544

_CACHE = {}


def _flat(ap3):
    """[p, c, n] tile view -> [p, c*n]."""
    return ap3.rearrange("p c n -> p (c n)")


def _ensure_scan_op():
    """Register the fused multiply + running-sum custom DVE op.

    out[p, k] = sum_{j<=k} in0[p, j] * in1[p, j]  (fp32 accumulate).
    One 1-elem/cycle pass does the wp (PSUM) multiply and the node
    reduction together; per-channel sums are recovered from the running
    sum at the node-block boundaries.
    """
    import concourse.dve_ops as dve_ops
    from concourse.dve_spec import AluOp, Spec, Src0, Src1, _has_src1, lower, scan
    from concourse.dve_uop import DveOpSpec

    name = "BONSAI_MUL_SCAN"
    for op in dve_ops.OPS:
        if op.name == name:
            return op

    def _ref(in0, in1, s0, s1, imm2):
        prod = in0.astype(np.float32) * np.asarray(in1).astype(np.float32)
        # the hardware scan runs over the flat stream order, crossing any
        # inner AP-dim boundaries
        flat = prod.reshape(prod.shape[0], -1)
        rs = np.cumsum(flat, axis=-1, dtype=np.float32)
        return rs.reshape(prod.shape)

    spec = Spec(body=scan(AluOp.ADD, Src0 * Src1), reference=_ref)
    row = max(dve_ops._SUB_OPCODE_FOR_NAME.values()) + 1
    assert row < 0x20
    dve_ops._SUB_OPCODE_FOR_NAME[name] = row
    shas = {}
    for ver in ("v3", "v4"):
        s = DveOpSpec(name=name, opcode=row, uops=lower(spec, ver=ver),
                      rd1_en=_has_src1(spec))
        shas[ver] = s.sha(ver)
    op = dve_ops.DveOp(name, spec, subdim=False, uops_sha=shas)
    dve_ops.OPS.append(op)
    dve_ops.CUSTOM_DVE_SPECS[name] = spec
    return op


def _build(tiles, group, fp32=True, mode=None):
    """Build the single-core Bass/Tile program for `tiles` 128-row tiles.

    mode: "fp32" | "bf16" | "scan".  "scan" uses the fused multiply +
    running-sum custom DVE op; the score DMA'd out is then the per-row
    cumulative channel sum and the host takes the channel diff.
    """
    from contextlib import ExitStack

    import concourse.bacc as bacc
    import concourse.bass as bass
    import concourse.mybir as mybir
    import concourse.tile as tile

    if mode is None:
        mode = "fp32" if fp32 else "bf16"
    fp32 = mode == "fp32"
    scan_op = _ensure_scan_op() if mode in ("scan", "scan2") else None

    f32 = mybir.dt.float32
    dt = f32 if fp32 else mybir.dt.bfloat16
    b_core = tiles * P

    nc = bacc.Bacc("TRN2", target_bir_lowering=False, debug=False)

    xT = nc.dram_tensor("xT", [P, 4, b_core], dt, kind="ExternalInput")
    Zs = nc.dram_tensor("Zs", [P, 4, PD], dt, kind="ExternalInput")
    T4T = nc.dram_tensor("T4T", [PD, 16], dt, kind="ExternalInput")
    Wcm = nc.dram_tensor("Wcm", [PD, FREE], dt, kind="ExternalInput")
    Vcm = nc.dram_tensor("Vcm", [PD, FREE], dt, kind="ExternalInput")
    score_d = nc.dram_tensor("score", [b_core, NCH], f32, kind="ExternalOutput")

    mult = mybir.AluOpType.mult
    Sigmoid = mybir.ActivationFunctionType.Sigmoid
    Tanh = mybir.ActivationFunctionType.Tanh

    import json
    import os
    cfg = json.loads(os.environ.get("BONSAI_CFG", "{}"))

    groups = []
    t0 = 0
    # ramp: small leading groups so the vector engine (the bottleneck)
    # starts its phase-3 work as early as possible instead of waiting
    # for a full first group's loads + projections
    ramp = [g for g in cfg.get("ramp", (2, 4, 6, 7)) if g < group]
    while t0 < tiles:
        g = min(ramp.pop(0) if ramp else group, tiles - t0)
        if mode == "scan2":
            assert g % 2 == 0, "scan2 needs even group sizes"
        groups.append((t0, g))
        t0 += g

    xbufs = cfg.get("xbufs", 8)
    vbufs = cfg.get("vbufs", 10)
    ppb = cfg.get("ppb", 2)
    tbb = cfg.get("tbb", 2)
    wvb = cfg.get("wvb", 1)

    with tile.TileContext(nc) as tc, ExitStack() as ctx:
        cpool = ctx.enter_context(tc.tile_pool(name="consts", bufs=1))
        xpool = ctx.enter_context(tc.tile_pool(name="xin", bufs=xbufs))
        ppool = ctx.enter_context(
            tc.tile_pool(name="ppt", bufs=group + cfg.get("ppx", 4)))
        gpool = ctx.enter_context(
            tc.tile_pool(name="grp", bufs=cfg.get("gbufs", 2)))
        vpool = ctx.enter_context(tc.tile_pool(name="elem", bufs=vbufs))
        spool = ctx.enter_context(
            tc.tile_pool(name="score", bufs=cfg.get("sbufs", 4)))
        if mode == "scan2":
            ppb = tbb = wvb = 1
        wpb = cfg.get("wpb", 1)
        if mode == "scan" and wpb > 1:
            # wp double-buffered so the next tile's wp matmuls overlap the
            # current scan; costs pp/tb their double buffers (PSUM = 8 banks)
            ppb = tbb = 1
        ps_pp = ctx.enter_context(
            tc.tile_pool(name="ps_pp", bufs=ppb, space=bass.MemorySpace.PSUM))
        ps_tb = ctx.enter_context(
            tc.tile_pool(name="ps_tb", bufs=tbb, space=bass.MemorySpace.PSUM))
        ps_wv = ctx.enter_context(
            tc.tile_pool(name="ps_wv", bufs=wvb, space=bass.MemorySpace.PSUM))
        ps_wp = (ctx.enter_context(
            tc.tile_pool(name="ps_wp", bufs=wpb, space=bass.MemorySpace.PSUM))
            if mode == "scan" and wpb > 1 else ps_wv)
        ps_w2 = (ctx.enter_context(
            tc.tile_pool(name="ps_w2", bufs=1, space=bass.MemorySpace.PSUM))
            if mode == "scan2" else None)

        Zs_sb = cpool.tile([P, 4, PD], dt)
        T4T_sb = cpool.tile([PD, 16], dt)
        Wcm_sb = cpool.tile([PD, FREE], dt)
        Vcm_sb = cpool.tile([PD, FREE], dt)
        zt_loaded = False
        wv_loaded = False

        # dummy activations at t~0 so the ACT spline-table loads (~1.3us
        # each) overlap the DMA-bound startup instead of stalling the
        # first real sigmoid/tanh mid-pipeline
        warm = cpool.tile([1, 2], f32)
        nc.gpsimd.memset(warm[:], 0.0)
        nc.scalar.activation(warm[:, 0:1], warm[:, 0:1], Sigmoid)
        nc.scalar.activation(warm[:, 1:2], warm[:, 1:2], Tanh)

        for (g0, gsz) in groups:
            # --- phase 1: load x tiles (paired DMAs), ppT, tree logits ---
            tb_ps = ps_tb.tile([P, gsz * 16], f32, tag="tb")
            ppts = []
            for ti0 in range(0, gsz, 2):
                npair = min(2, gsz - ti0)
                t = g0 + ti0
                xt = xpool.tile([P, 4, 2 * P], dt, tag="xt")
                nc.sync.dma_start(xt[:, :, :npair * P],
                                  xT.ap()[:, :, t * P:(t + npair) * P])
                if not zt_loaded:
                    # issued after the first x DMA: the serial HWDGE
                    # descriptor pipe (~625ns each) sits on the startup
                    # critical path
                    nc.sync.dma_start(Zs_sb[:], Zs.ap())
                    nc.sync.dma_start(T4T_sb[:], T4T.ap())
                    zt_loaded = True
                for k in range(npair):
                    ti = ti0 + k
                    pp_ps = ps_pp.tile([PD, P], f32, tag="pp")
                    for c in range(4):
                        nc.tensor.matmul(
                            pp_ps[:], Zs_sb[:, c, :],
                            xt[:, c, k * P:(k + 1) * P],
                            start=(c == 0), stop=(c == 3))
                    ppt = ppool.tile([PD, P], dt, tag="ppt")
                    nc.scalar.copy(ppt[:], pp_ps[:])
                    nc.tensor.matmul(
                        tb_ps[:, ti * 16:(ti + 1) * 16], ppt[:], T4T_sb[:],
                        start=True, stop=True)
                    ppts.append(ppt)
            if not wv_loaded:
                # emitted after the first tiles so the W/V loads don't
                # delay the startup-critical x DMAs in the queue
                nc.sync.dma_start(Wcm_sb[:], Wcm.ap())
                nc.sync.dma_start(Vcm_sb[:], Vcm.ap())
                wv_loaded = True

            # --- phase 2: indicators for the whole group ---
            # a_pm[:, g, j, 0] = sigmoid(+2 tb_j), [..., 1] = sigmoid(-2 tb_j)
            a_pm = gpool.tile([P, gsz, 16, 2], dt, tag="apm")
            tb3 = tb_ps[:].rearrange("p (g j) -> p g j", j=16)
            nc.scalar.activation(a_pm[:, :, :, 0], tb3, Sigmoid, scale=2.0)
            nc.scalar.activation(a_pm[:, :, :, 1], tb3, Sigmoid, scale=-2.0)
            I_g = gpool.tile([P, gsz, NODES], dt, tag="ig")
            nc.gpsimd.memset(I_g[:, :, 0], 1.0)
            nc.gpsimd.memset(I_g[:, :, NODES - 1], 0.0)
            # level-batched recursion: children of level-L parents j0..j1-1
            # are the contiguous nodes 2*j0+1 .. 2*j1, each parent value
            # broadcast over its (+,-) pair via a 0-stride trailing dim
            nc.vector.tensor_copy(I_g[:, :, 1:3], a_pm[:, :, 0, :])
            for (j0, j1) in ((1, 3), (3, 7), (7, 15)):
                par = I_g[:, :, j0:j1]
                par2 = bass.AP(par.tensor, par.offset, par.ap + [[0, 2]])
                nc.vector.tensor_mul(I_g[:, :, 2 * j0 + 1:2 * j1 + 1], par2,
                                     a_pm[:, :, j0:j1, :])

            # --- phase 3 (paired): two batch tiles per DVE op ---
            if mode == "scan2":
                for ti in range(0, gsz, 2):
                    t = g0 + ti
                    wp2 = ps_w2.tile([P, 2, NCH, NODES], f32, tag="wp2")
                    fw2 = wp2[:].rearrange("p k c n -> p (k c n)")
                    tv2 = vpool.tile([P, 2, NCH, NODES], dt, tag="tv2")
                    for k in (0, 1):
                        ppt = ppts[ti + k]
                        o = k * FREE
                        nc.tensor.matmul(fw2[:, o:o + 512], ppt[:],
                                         Wcm_sb[:, 0:512], start=True, stop=True)
                        nc.tensor.matmul(fw2[:, o + 512:o + 1024], ppt[:],
                                         Wcm_sb[:, 512:1024], start=True, stop=True)
                        vp_ps = ps_wv.tile([P, NCH, NODES], f32, tag="vp")
                        fv = _flat(vp_ps[:])
                        nc.tensor.matmul(fv[:, 0:512], ppt[:],
                                         Vcm_sb[:, 0:512], start=True, stop=True)
                        nc.tensor.matmul(fv[:, 512:1024], ppt[:],
                                         Vcm_sb[:, 512:1024], start=True, stop=True)
                        nc.scalar.activation(
                            tv2[:, k].rearrange("p c n -> p (c n)"), fv, Tanh)
                    ib = I_g[:, ti:ti + 2, :]
                    ib4 = bass.AP(ib.tensor, ib.offset,
                                  [ib.ap[0], ib.ap[1], [0, NCH], ib.ap[2]])
                    tvi2 = vpool.tile([P, 2, NCH, NODES], dt, tag="tvi2")
                    nc.vector.tensor_mul(tvi2[:], tv2[:], ib4)
                    rs2 = vpool.tile([P, 2, NCH, NODES], f32, tag="rs2")
                    nc.vector._custom_dve(
                        scan_op, out=rs2[:].rearrange("p k c n -> p (k c n)"),
                        in0=fw2,
                        in1=tvi2[:].rearrange("p k c n -> p (k c n)"))
                    sc2 = spool.tile([P, 2, NCH], f32, tag="sc2")
                    nc.vector.tensor_copy(sc2[:], rs2[:, :, :, NODES - 1])
                    od = score_d.ap()[t * P:(t + 2) * P, :]
                    nc.sync.dma_start(
                        od.rearrange("(k p) c -> p k c", k=2), sc2[:])
                continue

            # --- phase 3: wp/vp matmuls, elementwise, reduce, store ---
            sc2 = None
            for ti in range(gsz):
                t = g0 + ti
                ppt = ppts[ti]
                wp_ps = ps_wp.tile([P, NCH, NODES], f32, tag="wp")
                vp_ps = ps_wv.tile([P, NCH, NODES], f32, tag="vp")
                fw = _flat(wp_ps[:])
                fv = _flat(vp_ps[:])
                nc.tensor.matmul(fw[:, 0:512], ppt[:], Wcm_sb[:, 0:512],
                                 start=True, stop=True)
                nc.tensor.matmul(fw[:, 512:1024], ppt[:], Wcm_sb[:, 512:1024],
                                 start=True, stop=True)
                nc.tensor.matmul(fv[:, 0:512], ppt[:], Vcm_sb[:, 0:512],
                                 start=True, stop=True)
                nc.tensor.matmul(fv[:, 512:1024], ppt[:], Vcm_sb[:, 512:1024],
                                 start=True, stop=True)
                tv = vpool.tile([P, NCH, NODES], dt, tag="tv")
                nc.scalar.activation(_flat(tv[:]), fv, Tanh)
                # I broadcast over the channel (middle) dim via a 0-stride AP
                ib = I_g[:, ti, :]
                ib3 = bass.AP(ib.tensor, ib.offset,
                              [ib.ap[0], [0, NCH], ib.ap[1]])
                if mode == "scan":
                    tvi = vpool.tile([P, NCH, NODES], dt, tag="tvi")
                    nc.vector.tensor_mul(tvi[:], tv[:], ib3)
                    # the scan skips the zero pad node (31 real nodes per
                    # channel block): custom-DVE rate is 1 elem/cycle
                    # regardless of AP shape, so odd counts cost nothing
                    k = ti % 2
                    if k == 0:
                        sc2 = spool.tile([P, 2, NCH], f32, tag="sc2")
                    rs = vpool.tile([P, NCH, TOT_N], f32, tag="rs")
                    nc.vector._custom_dve(
                        scan_op, out=rs[:], in0=wp_ps[:, :, :TOT_N],
                        in1=tvi[:, :, :TOT_N])
                    nc.vector.tensor_copy(sc2[:, k, :],
                                          rs[:, :, TOT_N - 1])
                    if k == 1 or ti == gsz - 1:
                        npair = k + 1
                        t0 = g0 + ti - k
                        od = score_d.ap()[t0 * P:(t0 + npair) * P, :]
                        nc.sync.dma_start(
                            od.rearrange("(k p) c -> p k c", k=npair),
                            sc2[:, :npair, :])
                    continue
                sc = spool.tile([P, NCH], f32, tag="sc")
                if True:
                    prod1 = vpool.tile([P, NCH, NODES], dt, tag="p1")
                    if fp32:
                        nc.vector.tensor_mul(_flat(prod1[:]), fw, _flat(tv[:]))
                    else:
                        # cross wp PSUM->SBUF on ScalarE so the DVE multiplies
                        # run in the 2x bf16 mode (SBUF-only operands)
                        wp_sb = vpool.tile([P, NCH, NODES], dt, tag="wpsb")
                        nc.scalar.copy(_flat(wp_sb[:]), fw)
                        nc.vector.tensor_mul(_flat(prod1[:]), _flat(wp_sb[:]),
                                             _flat(tv[:]))
                    prod2 = vpool.tile([P, NCH, NODES], dt, tag="p2")
                    nc.vector.tensor_mul(prod2[:], prod1[:], ib3)
                    nc.vector.tensor_reduce(
                        sc[:], prod2[:], axis=mybir.AxisListType.X,
                        op=mybir.AluOpType.add)
                nc.sync.dma_start(score_d.ap()[t * P:(t + 1) * P, :], sc[:])

    nc.compile()
    return nc


def _prep_inputs(x, Z, T, V, W, fp32=True):
    """Host-side: shard + relayout inputs for the per-core program."""
    import ml_dtypes

    dt = np.float32 if fp32 else ml_dtypes.bfloat16
    xp = np.zeros((N_CORES * B_CORE, NF), np.float32)
    xp[:BATCH] = x
    xp = xp.reshape(N_CORES, B_CORE, NF)

    Zs = np.ascontiguousarray(
        (np.asarray(Z, np.float32) / PD).reshape(4, P, PD).transpose(1, 0, 2)
    ).astype(dt)
    T4T = np.zeros((PD, 16), np.float32)
    T4T[:, :INT_N] = (SIG * np.asarray(T, np.float32)).T
    T4T = T4T.astype(dt)
    Wcm = np.zeros((PD, NCH, NODES), np.float32)
    Wcm[:, :, :TOT_N] = np.asarray(W, np.float32).transpose(1, 2, 0)
    Wcm = Wcm.reshape(PD, FREE).astype(dt)
    Vcm = np.zeros((PD, NCH, NODES), np.float32)
    Vcm[:, :, :TOT_N] = (SIG * np.asarray(V, np.float32)).transpose(1, 2, 0)
    Vcm = Vcm.reshape(PD, FREE).astype(dt)

    in_maps = []
    for c in range(N_CORES):
        xT = np.ascontiguousarray(
            xp[c].T.reshape(4, P, B_CORE).transpose(1, 0, 2)).astype(dt)
        in_maps.append(
            {"xT": xT, "Zs": Zs, "T4T": T4T, "Wcm": Wcm, "Vcm": Vcm})
    return in_maps


def kernel(x, Z, T, V, W):
    import os

    from concourse.bass_utils import run_bass_kernel_spmd

    mode = os.environ.get("BONSAI_MODE", "scan")
    try:
        key = ("nc", TILES_FULL, mode)
        if key not in _CACHE:
            _CACHE[key] = _build(TILES_FULL, 8, mode=mode)
        nc = _CACHE[key]
        in_maps = _prep_inputs(x, Z, T, V, W, fp32=(mode == "fp32"))
        res = run_bass_kernel_spmd(
            nc, in_maps, list(range(N_CORES)),
            trace=bool(os.environ.get("BONSAI_TRACE")))
    except Exception:
        if mode != "scan":
            raise
        # fall back to the stock-op pipeline if the custom-DVE path is
        # unavailable in this environment
        mode = "bf16"
        key = ("nc", TILES_FULL, mode)
        if key not in _CACHE:
            _CACHE[key] = _build(TILES_FULL, 8, mode=mode)
        nc = _CACHE[key]
        in_maps = _prep_inputs(x, Z, T, V, W, fp32=False)
        res = run_bass_kernel_spmd(
            nc, in_maps, list(range(N_CORES)),
            trace=bool(os.environ.get("BONSAI_TRACE")))
    _CACHE["last_result"] = res
    out = np.concatenate([r["score"] for r in res.results], axis=0)
    if mode in ("scan", "scan2"):
        raw = out.copy()
        out[:, 1:] = raw[:, 1:] - raw[:, :-1]
        if mode == "scan2":
            # odd tiles continue the even partner tile's running sum
            v_out = out.reshape(-1, 2, P, NCH)
            v_raw = raw.reshape(-1, 2, P, NCH)
            v_out[:, 1, :, 0] = v_raw[:, 1, :, 0] - v_raw[:, 0, :, NCH - 1]
    return np.ascontiguousarray(out[:BATCH]).astype(np.float32)

